# revision 1
# baseline (speedup 1.0000x reference)
"""GCN layer on 8 Trainium2 NeuronCores (Bass/Tile, SPMD).

  H' = X @ W^T                                  (dense projection, fp16)
  out[r] = sum_{e: row[e]==r} val[e] * H'[col[e]]  +  deg[r] * b
  where deg[r] = sum_{e: row[e]==r} val[e]     (bias folded via degree)

Sharding: destination nodes (rows of the output) are sharded across the
8 cores (12500 rows each); W/b replicated; each core computes the full
projection H' (replicated) into its own HBM, then gathers source rows
for its edge shard with SWDGE dma_gather and reduces them per 128-row
destination group with a selector-matrix matmul accumulated in PSUM.

Device-side layout:
  * Nodes are assigned to 4 chunks by n % 4 and renumbered q = n // 4.
    Chunk c's projection occupies a contiguous range of row-tiles, is
    written to its own DRAM buffer H_c[p, u, f] = H'[node(c, u*128+p)]
    (so PSUM tiles map 1:1 to large line-rate writes), and unblocks that
    chunk's gathers while later chunks are still projecting.
  * dma_gather indices are int16: the in-chunk index
    (q%128)*T_C + q//128 < 128*T_C = 25088 always fits.
  * Edges are bucketed by (dest-group g, chunk c); each bucket is padded
    to a multiple of 128 with val=0 copies of its last edge; the tile
    count per bucket is the max across all 8 cores so one SPMD program
    serves every core.
  * A 128-edge tile is reduced into its destination group's PSUM via
    matmul(lhsT=S_T, rhs=M) with S_T[e, r] = (lr[e] == r) built on DVE
    by one is_equal against a broadcast iota per group; edge values are
    folded into the gathered messages with one multiply per section.
  * Each group's PSUM accumulation starts with a rank-1 matmul
    deg_g (x) b that contributes the bias term.
"""

import numpy as np

from concourse import bacc, mybir, tile
from concourse.bass_utils import run_bass_kernel_spmd

dt = mybir.dt

# ---------------------------------------------------------------- constants
N_NODES = 100000
IN_DIM = 256
OUT_DIM = 128
N_EDGES = 1600000
N_CORES = 8
P = 128
NCH = 4


class Cfg:
    def __init__(self, n_nodes=N_NODES, rows_per_core=12500, block_groups=8,
                 max_gather_slots=8, n_queues=4, dma_scratch=16384,
                 trim_pads=False):
        self.n_nodes = n_nodes
        self.rows_per_core = rows_per_core
        self.max_gather_slots = max_gather_slots
        self.n_queues = n_queues
        self.dma_scratch = dma_scratch
        self.trim_pads = trim_pads
        self.chunk_nodes = -(-n_nodes // NCH)        # nodes per chunk (n % 4)
        self.t_c = -(-self.chunk_nodes // P)         # row tiles per chunk
        self.chunk_rows_pad = self.t_c * P
        assert self.chunk_rows_pad < 32768, "int16 gather index overflow"
        self.t_t = NCH * self.t_c                    # total projection tiles
        self.n_nodes_pad = self.t_t * P
        self.ng = -(-rows_per_core // P)             # dest groups per core
        self.rows_pad = self.ng * P
        self.block_groups = block_groups


DEFAULT_CFG = Cfg()


# ---------------------------------------------------------------- host side
def build_schedule(cfg, edge_row, edge_col, edge_val):
    """Bucket edges by (core, dest-group, chunk); build the shared SPMD
    schedule (cross-core max tile counts) and per-core data arrays."""
    er = np.asarray(edge_row).astype(np.int64)
    ec = np.asarray(edge_col).astype(np.int64)
    ev = np.asarray(edge_val).astype(np.float32)

    core = er // cfg.rows_per_core
    lr = er - core * cfg.rows_per_core
    g = lr // P
    lrg = (lr - g * P).astype(np.float16)        # in-group row, exact in fp16
    c = ec % NCH
    q = ec // NCH
    idx16 = ((q % P) * cfg.t_c + q // P).astype(np.int16)
    val16 = ev.astype(np.float16)

    ng = cfg.ng
    bucket = ((core * ng + g) * NCH + c).astype(np.int64)
    order = np.argsort(bucket, kind="stable")
    cnt = np.bincount(bucket, minlength=N_CORES * ng * NCH).reshape(N_CORES, ng, NCH)
    starts = np.zeros((N_CORES, ng, NCH), dtype=np.int64)
    np.cumsum(cnt.reshape(-1)[:-1], out=starts.reshape(-1)[1:])

    idx16_s = idx16[order]
    lrg_s = lrg[order]
    val16_s = val16[order]

    T = np.ceil(cnt / P).astype(np.int64).max(axis=0)      # [ng, NCH]

    blocks = [list(range(b, min(b + cfg.block_groups, ng)))
              for b in range(0, ng, cfg.block_groups)]
    tot_slots = int(T.sum())
    tot8 = tot_slots * 8
    g_tiles = T.sum(axis=1)
    g_qoff = np.zeros(ng, dtype=np.int64)
    np.cumsum(g_tiles[:-1], out=g_qoff[1:])

    # section = (block, chunk): contiguous slot range in the global arrays
    slot_cursor = 0
    blk_meta = []
    for blk in blocks:
        secs = []
        blk_slot0 = slot_cursor
        for cc in range(NCH):
            n_slots = int(T[blk, cc].sum())
            secs.append(dict(c=cc, n_slots=n_slots, slot0=slot_cursor,
                             col8_off=slot_cursor * 8))
            slot_cursor += n_slots
        # per-group: list of (c, section-local slot, k-index in group-major S)
        gmap = {}
        for gi in blk:
            ents = []
            kk = 0
            for cc in range(NCH):
                o_gc = int(T[[x for x in blk if x < gi], cc].sum())
                for k in range(int(T[gi, cc])):
                    ents.append((cc, o_gc + k))
                    kk += 1
            gmap[gi] = ents
        blk_meta.append(dict(blk=blk, secs=secs, blk_slot0=blk_slot0,
                             n_slots=slot_cursor - blk_slot0, gmap=gmap))
    assert slot_cursor == tot_slots
    sec_max = max((s["n_slots"] for bm in blk_meta for s in bm["secs"]),
                  default=1)

    cols = np.zeros((N_CORES, 16, tot8), dtype=np.int16)
    lrg_arr = np.zeros((N_CORES, P, tot_slots), dtype=np.float16)   # group-major
    val_arr = np.zeros((N_CORES, P, tot_slots), dtype=np.float16)   # slot-major
    deg_arr = np.zeros((N_CORES, 1, cfg.rows_pad), dtype=np.float16)

    for ci in range(N_CORES):
        m = core == ci
        deg = np.bincount(lr[m], weights=ev[m], minlength=cfg.rows_pad)
        deg_arr[ci, 0] = deg.astype(np.float16)
        sec_idx = 0
        for bm in blk_meta:
            for sec in bm["secs"]:
                cc, n_slots = sec["c"], sec["n_slots"]
                if n_slots == 0:
                    continue
                trim = cfg.trim_pads and sec_idx >= 6
                sec_idx += 1
                parts_idx, parts_val, parts_lrg = [], [], []
                for gi in bm["blk"]:
                    n = int(cnt[ci, gi, cc])
                    s0 = int(starts[ci, gi, cc])
                    tgt = int(T[gi, cc]) * P
                    bi_ = idx16_s[s0:s0 + n]
                    bv = val16_s[s0:s0 + n]
                    bl = lrg_s[s0:s0 + n]
                    if tgt > n:
                        if trim:
                            pad_idx = np.int16(-1)
                        else:
                            pad_idx = bi_[-1] if n > 0 else np.int16(0)
                        bi_ = np.concatenate([bi_, np.full(tgt - n, pad_idx, np.int16)])
                        bv = np.concatenate([bv, np.zeros(tgt - n, np.float16)])
                        bl = np.concatenate([bl, np.zeros(tgt - n, np.float16)])
                    parts_idx.append(bi_)
                    parts_val.append(bv)
                    parts_lrg.append(bl)
                sidx = np.concatenate(parts_idx)
                sval = np.concatenate(parts_val)
                slrg = np.concatenate(parts_lrg)
                n_e = n_slots * P
                assert sidx.shape[0] == n_e
                cols[ci, :, sec["col8_off"]:sec["col8_off"] + n_slots * 8] = \
                    sidx.reshape(n_e // 16, 16).T
                val_arr[ci, :, sec["slot0"]:sec["slot0"] + n_slots] = \
                    sval.reshape(n_slots, P).T
                qo = 0
                for gi in bm["blk"]:
                    tg = int(T[gi, cc]) * P
                    qcol = g_qoff[gi] + int(T[gi, :cc].sum())
                    lrg_arr[ci, :, qcol:qcol + tg // P] = \
                        slrg[qo:qo + tg].reshape(tg // P, P).T
                    qo += tg

    cols_full = np.tile(cols, (1, 8, 1))

    sched = dict(T=T, blocks=blocks, blk_meta=blk_meta, tot_slots=tot_slots,
                 tot8=tot8, g_qoff=g_qoff, g_tiles=g_tiles, sec_max=sec_max,
                 tg_max=int(g_tiles.max()) if ng else 1)
    data = dict(cols=cols_full, lrg=lrg_arr, val=val_arr, deg=deg_arr)
    return sched, data


# ---------------------------------------------------------------- device side
def build_program(cfg, sched):
    nc = bacc.Bacc("TRN2", target_bir_lowering=False, debug=False,
                   num_swdge_queues=cfg.n_queues,
                   dynamic_dma_scratch_size=cfg.dma_scratch)
    f16, f32 = dt.float16, dt.float32

    XT = nc.dram_tensor("xt", [IN_DIM, cfg.n_nodes_pad], f16, kind="ExternalInput")
    WT = nc.dram_tensor("wt", [IN_DIM, OUT_DIM], f16, kind="ExternalInput")
    BR = nc.dram_tensor("brow", [1, OUT_DIM], f16, kind="ExternalInput")
    IO = nc.dram_tensor("iota", [P, P], f16, kind="ExternalInput")
    CO = nc.dram_tensor("cols", [P, sched["tot8"]], dt.int16, kind="ExternalInput")
    LG = nc.dram_tensor("lrg", [P, sched["tot_slots"]], f16, kind="ExternalInput")
    VA = nc.dram_tensor("val", [P, sched["tot_slots"]], f16, kind="ExternalInput")
    DG = nc.dram_tensor("deg", [1, cfg.rows_pad], f16, kind="ExternalInput")
    OUT = nc.dram_tensor("out", [cfg.rows_pad, OUT_DIM], f32, kind="ExternalOutput")

    T = sched["T"]
    blk_meta = sched["blk_meta"]
    t_c = cfg.t_c

    with tile.TileContext(nc) as tc:
        with (
            tc.tile_pool(name="dram", bufs=1, space="DRAM") as dpool,
            tc.tile_pool(name="const", bufs=1) as cpool,
            tc.tile_pool(name="spsum", bufs=4, space="PSUM") as spp,
        ):
            H = [dpool.tile([P, t_c, OUT_DIM], f16, tag=f"h{c}", name=f"hbuf{c}")
                 for c in range(NCH)]
            H_flat = [h[:].rearrange("p t f -> (p t) f") for h in H]

            wt0 = cpool.tile([P, OUT_DIM], f16)
            wt1 = cpool.tile([P, OUT_DIM], f16)
            nc.sync.dma_start(out=wt0[:], in_=WT[0:P, :])
            nc.sync.dma_start(out=wt1[:], in_=WT[P:2 * P, :])
            brow_t = cpool.tile([1, OUT_DIM], f16)
            nc.sync.dma_start(out=brow_t[:], in_=BR[:, :])
            iota_t = cpool.tile([P, P], f16)
            nc.sync.dma_start(out=iota_t[:], in_=IO[:, :])

            # ---------------- phase 1: H' = X @ W^T, chunk by chunk
            QUAD = 4
            SLAB = 98             # row tiles per XT slab DMA (t_c = 196 = 2*98)
            with (
                tc.tile_pool(name="xt", bufs=2) as xtp,
                tc.tile_pool(name="hpsum", bufs=2, space="PSUM") as hpp,
                tc.tile_pool(name="hstg", bufs=3) as hsp,
            ):
                for s0 in range(0, cfg.t_t, SLAB):
                    s1 = min(s0 + SLAB, cfg.t_t)
                    rows = (s1 - s0) * P
                    xt0 = xtp.tile([P, SLAB * P], f16, tag="x0")
                    xt1 = xtp.tile([P, SLAB * P], f16, tag="x1")
                    nc.sync.dma_start(out=xt0[:, :rows], in_=XT[0:P, s0 * P:s1 * P])
                    nc.sync.dma_start(out=xt1[:, :rows], in_=XT[P:2 * P, s0 * P:s1 * P])
                    q0 = s0
                    while q0 < s1:
                        cc = q0 // t_c
                        q1 = min(q0 + QUAD, s1, (cc + 1) * t_c)
                        nq = q1 - q0
                        ps = hpp.tile([P, QUAD * OUT_DIM], f32)
                        for qi, t in enumerate(range(q0, q1)):
                            o = (t - s0) * P
                            seg = ps[:, qi * OUT_DIM:(qi + 1) * OUT_DIM]
                            nc.tensor.matmul(seg, lhsT=xt0[:, o:o + P], rhs=wt0[:],
                                             start=True, stop=False)
                            nc.tensor.matmul(seg, lhsT=xt1[:, o:o + P], rhs=wt1[:],
                                             start=False, stop=True)
                        hst = hsp.tile([P, QUAD, OUT_DIM], f16)
                        nc.any.tensor_copy(
                            hst[:, :nq, :],
                            ps[:, :nq * OUT_DIM].rearrange("p (q f) -> p q f", f=OUT_DIM))
                        u0 = q0 % t_c
                        assert (q1 - 1) // t_c == cc
                        nc.sync.dma_start(out=H[cc][:, u0:u0 + nq, :],
                                          in_=hst[:, :nq, :])
                        q0 = q1

            # ---------------- phase 2: gather + selector-matmul segment sum
            with (
                tc.tile_pool(name="sec", bufs=6) as secp,
                tc.tile_pool(name="side", bufs=2) as sidep,
                tc.tile_pool(name="st", bufs=10) as stp,
                tc.tile_pool(name="ostg", bufs=3) as opool,
            ):
                gq = 0
                for bi, bm in enumerate(blk_meta):
                    nsl = bm["n_slots"]
                    blk = bm["blk"]
                    nbg = len(blk)
                    blk_slot0 = bm["blk_slot0"]
                    g0 = blk[0]
                    q_lo = int(sched["g_qoff"][g0])
                    q_hi = q_lo + int(sum(sched["g_tiles"][gi] for gi in blk))
                    # per-block sideband loads
                    lrg_t = sidep.tile([P, sched["tg_max"] * cfg.block_groups],
                                       f16, tag="lrg")
                    val_t = sidep.tile([P, sched["sec_max"] * NCH], f16, tag="val")
                    cols_t = sidep.tile([P, sched["sec_max"] * NCH * 8], dt.int16,
                                        tag="cols")
                    deg_t = sidep.tile([1, cfg.block_groups * P], f16, tag="deg")
                    if q_hi > q_lo:
                        nc.sync.dma_start(out=lrg_t[:, :q_hi - q_lo],
                                          in_=LG[:, q_lo:q_hi])
                    if nsl:
                        nc.sync.dma_start(out=val_t[:, :nsl],
                                          in_=VA[:, blk_slot0:blk_slot0 + nsl])
                        nc.sync.dma_start(
                            out=cols_t[:, :nsl * 8],
                            in_=CO[:, blk_slot0 * 8:(blk_slot0 + nsl) * 8])
                    nc.sync.dma_start(out=deg_t[:, :nbg * P],
                                      in_=DG[:, g0 * P:g0 * P + nbg * P])

                    # S_T per group (only needs lrg/iota — can run early)
                    st_tiles = {}
                    for gi in blk:
                        tg = int(sched["g_tiles"][gi])
                        st = stp.tile([P, sched["tg_max"] * P], f16, tag="st")
                        st_tiles[gi] = (st, tg)
                        if tg == 0:
                            continue
                        ql = int(sched["g_qoff"][gi]) - q_lo
                        nc.vector.tensor_tensor(
                            out=st[:, :tg * P].rearrange("p (t r) -> p t r", r=P),
                            in0=lrg_t[:, ql:ql + tg, None].to_broadcast([P, tg, P]),
                            in1=iota_t[:, None, :].to_broadcast([P, tg, P]),
                            op=mybir.AluOpType.is_equal,
                        )

                    # per-chunk sections: gather -> val multiply
                    mt_tiles = {}
                    for sec in bm["secs"]:
                        cc, sns = sec["c"], sec["n_slots"]
                        if sns == 0:
                            continue
                        soff = sec["slot0"] - blk_slot0
                        mt = secp.tile([P, sched["sec_max"], OUT_DIM], f16, tag="m")
                        mt_tiles[cc] = mt
                        for a in range(0, sns, cfg.max_gather_slots):
                            k = min(cfg.max_gather_slots, sns - a)
                            nc.gpsimd.dma_gather(
                                out_ap=mt[:, a:a + k, :],
                                in_ap=H_flat[cc],
                                idxs_ap=cols_t[:, (soff + a) * 8:(soff + a + k) * 8],
                                num_idxs=k * P,
                                num_idxs_reg=k * P,
                                elem_size=OUT_DIM,
                                queue_num=gq % cfg.n_queues,
                            )
                            gq += 1
                        nc.any.tensor_tensor(
                            out=mt[:, :sns, :],
                            in0=mt[:, :sns, :],
                            in1=val_t[:, soff:soff + sns, None]
                            .to_broadcast([P, sns, OUT_DIM]),
                            op=mybir.AluOpType.mult,
                        )

                    # matmuls: per group, one contiguous accumulation chain
                    psums = {}
                    for pi in range(0, nbg, 4):
                        ps = spp.tile([P, 4 * OUT_DIM], f32)
                        for gi in blk[pi:pi + 4]:
                            j = blk.index(gi)
                            psums[gi] = ps[:, (j % 4) * OUT_DIM:(j % 4 + 1) * OUT_DIM]
                            tg = int(sched["g_tiles"][gi])
                            nc.tensor.matmul(
                                psums[gi],
                                lhsT=deg_t[0:1, j * P:(j + 1) * P],
                                rhs=brow_t[0:1, :],
                                start=True, stop=tg == 0)
                            st, _tg = st_tiles[gi]
                            kk = 0
                            for cc in range(NCH):
                                o_gc = int(T[[x for x in blk if x < gi], cc].sum())
                                n_t = int(T[gi, cc])
                                for k in range(n_t):
                                    nc.tensor.matmul(
                                        psums[gi],
                                        lhsT=st[:, kk * P:(kk + 1) * P],
                                        rhs=mt_tiles[cc][:, o_gc + k, :],
                                        start=False,
                                        stop=kk == tg - 1,
                                    )
                                    kk += 1

                    # evict: psum -> sbuf f32 -> OUT
                    for pi in range(0, nbg, 4):
                        npg = min(4, nbg - pi)
                        og = opool.tile([P, 4, OUT_DIM], f32)
                        for j in range(npg):
                            nc.any.tensor_copy(og[:, j, :], psums[blk[pi + j]])
                        nc.sync.dma_start(
                            out=OUT[blk[pi] * P:(blk[pi] + npg) * P, :]
                            .rearrange("(q p) f -> p q f", p=P),
                            in_=og[:, :npg, :])

    nc.finalize()
    return nc


# ---------------------------------------------------------------- driver
_CACHE = {}


def _patch_walrus_ldw_opt():
    """Enable walrus LDWEIGHTS double-buffering (off by default)."""
    import concourse.bass_utils as _bu
    if getattr(_bu, "_ldw_patched", False):
        return
    orig = _bu.run_command

    def run_command2(argv, **kw):
        argv = ["--enable-ldw-opt=true" if a == "--enable-ldw-opt=false" else a
                for a in argv]
        return orig(argv, **kw)

    _bu.run_command = run_command2
    _bu._ldw_patched = True


def _ensure_ntff_hook():
    """Provide antenv.axon_hooks + the ctypes NTFF profile hook when the
    agent image lacks them (needed only for trace=True)."""
    import sys
    import types
    import contextlib
    import ctypes
    try:
        from antenv.axon_hooks import get_axon_ntff_profile_hook  # noqa: F401
        return
    except ImportError:
        pass
    import antenv
    mod = types.ModuleType("antenv.axon_hooks")
    mod._hook = None

    def set_axon_ntff_profile_hook(h):
        mod._hook = h

    def get_axon_ntff_profile_hook():
        return mod._hook

    mod.set_axon_ntff_profile_hook = set_axon_ntff_profile_hook
    mod.get_axon_ntff_profile_hook = get_axon_ntff_profile_hook
    sys.modules["antenv.axon_hooks"] = mod
    antenv.axon_hooks = mod

    so_path = "/opt/axon/libaxon_pjrt.so"
    try:
        lib = ctypes.CDLL(so_path)
    except OSError:
        return
    if not hasattr(lib, "axon_start_nrt_profile"):
        return
    lib.axon_start_nrt_profile.argtypes = [ctypes.POINTER(ctypes.c_int64),
                                           ctypes.c_size_t]
    lib.axon_start_nrt_profile.restype = ctypes.c_int64
    lib.axon_stop_nrt_profile.argtypes = [ctypes.c_char_p]
    lib.axon_stop_nrt_profile.restype = ctypes.c_int64

    @contextlib.contextmanager
    def _hook(output_dir, device_ids):
        import jax
        jax.devices()
        if device_ids:
            ids = (ctypes.c_int64 * len(device_ids))(*device_ids)
            rc = lib.axon_start_nrt_profile(ids, len(device_ids))
        else:
            rc = lib.axon_start_nrt_profile(None, 0)
        if rc != 0:
            raise RuntimeError(f"axon_start_nrt_profile rc={rc}")
        try:
            yield
        finally:
            n = lib.axon_stop_nrt_profile(str(output_dir).encode())
            print(f"ntff profile: {n} file(s) written to {output_dir}", flush=True)

    set_axon_ntff_profile_hook(_hook)


def _prep_inputs(cfg, X, W, b, sched, data):
    Xf = np.asarray(X, np.float32).astype(np.float16)
    n = Xf.shape[0]
    xt = np.zeros((IN_DIM, cfg.n_nodes_pad), dtype=np.float16)
    narr = np.arange(n)
    cch = narr % NCH
    qq = narr // NCH
    col = (cch * cfg.t_c + qq // P) * P + (qq % P)
    xt[:, col] = Xf.T
    wt = np.asarray(W, np.float32).T.astype(np.float16)
    brow = np.asarray(b, np.float32).astype(np.float16)[None, :]
    iota = np.tile(np.arange(P, dtype=np.float16)[None, :], (P, 1))
    in_maps = []
    for ci in range(N_CORES):
        in_maps.append({
            "xt": xt, "wt": wt, "brow": brow, "iota": iota,
            "cols": data["cols"][ci], "lrg": data["lrg"][ci],
            "val": data["val"][ci], "deg": data["deg"][ci],
        })
    return in_maps


def run(X, edge_row, edge_col, edge_val, W, b, cfg=DEFAULT_CFG, trace=False):
    if trace:
        _ensure_ntff_hook()
    sched, data = build_schedule(cfg, edge_row, edge_col, edge_val)
    key = ("prog", cfg.n_nodes, cfg.rows_per_core, sched["tot_slots"],
           tuple(sched["T"].reshape(-1)))
    if key not in _CACHE:
        _CACHE.clear()
        _CACHE[key] = build_program(cfg, sched)
    nc = _CACHE[key]
    in_maps = _prep_inputs(cfg, X, W, b, sched, data)
    res = run_bass_kernel_spmd(nc, in_maps, core_ids=list(range(N_CORES)),
                               trace=trace)
    outs = [res.results[ci]["out"][:cfg.rows_per_core] for ci in range(N_CORES)]
    full = np.concatenate(outs, axis=0).astype(np.float32)
    return full, res


def kernel(X, edge_row, edge_col, edge_val, W, b):
    out, _ = run(X, edge_row, edge_col, edge_val, W, b)
    return out



# revision 12
# speedup vs baseline: 1.1430x; 1.1430x over previous
"""GCN layer on 8 Trainium2 NeuronCores (Bass/Tile, SPMD).

  H' = X @ W^T                                  (dense projection, fp16)
  out[r] = sum_{e: row[e]==r} val[e] * H'[col[e]]  +  deg[r] * b
  where deg[r] = sum_{e: row[e]==r} val[e]     (bias folded via degree)

Sharding: destination nodes (rows of the output) are sharded across the
8 cores (12500 rows each); W/b replicated; each core computes the full
projection H' (replicated) into its own HBM, then gathers source rows
for its edge shard with SWDGE dma_gather and reduces them per 128-row
destination group with a selector-matrix matmul accumulated in PSUM.

Pipelined chunk-major structure (v2):
  * Nodes are assigned to 4 chunks by n % 4 and renumbered q = n // 4.
    Chunk c's projection is written to its own DRAM buffer
    H_c[p, u, f] = H'[node(c, u*128+p)].
  * Edges are bucketed by (core, dest-group g, chunk c), padded to a
    multiple of 128 using the cross-core max tile count T[g,c] so one
    SPMD program serves every core.  Sidebands are laid out ROUND-major
    (chunk, block, group) so each round's data is contiguous.
  * Work proceeds in 4 rounds, one per chunk: round c projects chunk c
    (tensor+DMA) while gathers / selector builds / matmuls of round c-1
    run on gpsimd/vector/tensor.  Per (block, round): PSUM accumulates
    bias (round 0) or the re-injected fp16 SBUF accumulator (identity
    matmul), plus the round's selector matmuls; then PSUM is evicted
    back to the fp16 accumulator (scalar engine).  This keeps PSUM
    lifetimes short while letting every engine start as soon as its
    chunk's data exists.
  * val[e] is folded into the selector matrix S_T (not the messages), so
    gathered messages go straight from DMA into the matmul.
  * S_T build runs in the DVE 2x packed-fp16 mode: the destination-row
    compare uses a host-side 8x-replicated lrg8 sideband and a 4D
    [P, slots, 16, 8] access pattern so every operand's innermost axis
    is stride-1 (broadcasts on middle axes only).  Same for the val fold
    (val8 sideband).
  * Bucket pads use idx=-1 which dma_gather skips (no DMA traffic); the
    corresponding S_T rows are 0 (val pad = 0) so stale SBUF contributes
    nothing.  The first MT_BUFS sections use duplicate-index pads
    instead so fresh (uninitialized) SBUF is never multiplied.
"""

import numpy as np

from concourse import bacc, mybir, tile
from concourse.bass_utils import run_bass_kernel_spmd

dt = mybir.dt

# ---------------------------------------------------------------- constants
N_NODES = 100000
IN_DIM = 256
OUT_DIM = 128
N_EDGES = 1600000
N_CORES = 8
P = 128
NCH = 4


class Cfg:
    def __init__(self, n_nodes=N_NODES, rows_per_core=12500, block_groups=8,
                 n_queues=4, dma_scratch=16384, mt_bufs=4, st_bufs=3,
                 slab=14):
        self.n_nodes = n_nodes
        self.rows_per_core = rows_per_core
        self.n_queues = n_queues
        self.dma_scratch = dma_scratch
        self.mt_bufs = mt_bufs
        self.st_bufs = st_bufs
        self.slab = slab
        self.chunk_nodes = -(-n_nodes // NCH)        # nodes per chunk (n % 4)
        self.t_c = -(-self.chunk_nodes // P)         # row tiles per chunk
        self.chunk_rows_pad = self.t_c * P
        assert self.chunk_rows_pad < 32768, "int16 gather index overflow"
        self.t_t = NCH * self.t_c                    # total projection tiles
        self.n_nodes_pad = self.t_t * P
        self.ng = -(-rows_per_core // P)             # dest groups per core
        self.rows_pad = self.ng * P
        self.block_groups = block_groups
        assert self.t_c % self.slab == 0


DEFAULT_CFG = Cfg()


# ---------------------------------------------------------------- host side
def build_schedule(cfg, edge_row, edge_col, edge_val):
    """Bucket edges by (core, dest-group, chunk); build the shared SPMD
    schedule (cross-core max tile counts, round-major layout) and the
    per-core data arrays."""
    er = np.asarray(edge_row).astype(np.int64)
    ec = np.asarray(edge_col).astype(np.int64)
    ev = np.asarray(edge_val).astype(np.float32)

    core = er // cfg.rows_per_core
    lr = er - core * cfg.rows_per_core
    g = lr // P
    lrg = (lr - g * P).astype(np.float16)        # in-group row, exact in fp16
    c = ec % NCH
    q = ec // NCH
    idx16 = ((q % P) * cfg.t_c + q // P).astype(np.int16)
    val16 = ev.astype(np.float16)

    ng = cfg.ng
    bucket = ((core * ng + g) * NCH + c).astype(np.int64)
    order = np.argsort(bucket, kind="stable")
    cnt = np.bincount(bucket, minlength=N_CORES * ng * NCH).reshape(N_CORES, ng, NCH)
    starts = np.zeros((N_CORES, ng, NCH), dtype=np.int64)
    np.cumsum(cnt.reshape(-1)[:-1], out=starts.reshape(-1)[1:])

    idx16_s = idx16[order]
    lrg_s = lrg[order]
    val16_s = val16[order]

    T = np.maximum(np.ceil(cnt / P).astype(np.int64).max(axis=0), 1)  # [ng, NCH]
    assert T.max() <= 8, f"bucket too large for one gather call: {T.max()}"
    # cross-core max real count per bucket: num_idxs_reg must equal the
    # per-core count of non-negative indices, and the SPMD program shares
    # one constant -- so every core dup-pads its bucket up to M[g,c].
    M = np.maximum(cnt.max(axis=0), 1)                                # [ng, NCH]

    blocks = [list(range(b, min(b + cfg.block_groups, ng)))
              for b in range(0, ng, cfg.block_groups)]

    # round-major sections: (chunk, block) -> contiguous slot range
    sec_meta = []
    slot_cursor = 0
    for cc in range(NCH):
        for bi, blk in enumerate(blocks):
            o = {}
            k = 0
            for gi in blk:
                o[gi] = k
                k += int(T[gi, cc])
            sec_meta.append(dict(c=cc, bi=bi, slot0=slot_cursor, nsl=k, o=o))
            slot_cursor += k
    tot_slots = slot_cursor
    tot8 = tot_slots * 8
    nsl_max = max(sm["nsl"] for sm in sec_meta)

    cols = np.full((N_CORES, 16, tot8), -1, dtype=np.int16)
    l8 = np.zeros((N_CORES, P, tot_slots, 8), dtype=np.float16)
    v8 = np.zeros((N_CORES, P, tot_slots, 8), dtype=np.float16)
    deg_arr = np.zeros((N_CORES, 1, cfg.rows_pad), dtype=np.float16)

    fresh_skip = cfg.mt_bufs   # first sections use fresh SBUF: no -1 trims

    for ci in range(N_CORES):
        m = core == ci
        deg = np.bincount(lr[m], weights=ev[m], minlength=cfg.rows_pad)
        deg_arr[ci, 0] = deg.astype(np.float16)
        for si, sm in enumerate(sec_meta):
            cc = sm["c"]
            trim = si >= fresh_skip
            for gi in blocks[sm["bi"]]:
                n = int(cnt[ci, gi, cc])
                s0 = int(starts[ci, gi, cc])
                tgt = int(T[gi, cc]) * P
                bi_ = idx16_s[s0:s0 + n]
                bv = val16_s[s0:s0 + n]
                bl = lrg_s[s0:s0 + n]
                if tgt > n:
                    # valid (dup-idx) pads up to mm, then -1 (skipped by the
                    # gather) up to the 128-multiple tgt
                    mm = tgt if not trim else int(M[gi, cc])
                    dup = bi_[-1] if n > 0 else np.int16(0)
                    pads = np.full(tgt - n, np.int16(-1))
                    pads[:mm - n] = dup
                    bi_ = np.concatenate([bi_, pads])
                    bv = np.concatenate([bv, np.zeros(tgt - n, np.float16)])
                    bl = np.concatenate([bl, np.zeros(tgt - n, np.float16)])
                gslot = sm["slot0"] + sm["o"][gi]
                tg = int(T[gi, cc])
                cols[ci, :, gslot * 8:(gslot + tg) * 8] = \
                    bi_.reshape(tgt // 16, 16).T
                l8[ci, :, gslot:gslot + tg, :] = \
                    bl.reshape(tg, P).T[:, :, None]
                v8[ci, :, gslot:gslot + tg, :] = \
                    bv.reshape(tg, P).T[:, :, None]

    cols_full = np.tile(cols, (1, 8, 1))
    sched = dict(T=T, M=M, blocks=blocks, sec_meta=sec_meta,
                 tot_slots=tot_slots, tot8=tot8, nsl_max=nsl_max,
                 fresh_skip=fresh_skip)
    data = dict(cols=cols_full,
                l8=l8.reshape(N_CORES, P, tot_slots * 8),
                v8=v8.reshape(N_CORES, P, tot_slots * 8),
                deg=deg_arr)
    return sched, data


# ---------------------------------------------------------------- device side
def build_program(cfg, sched):
    nc = bacc.Bacc("TRN2", target_bir_lowering=False, debug=False,
                   num_swdge_queues=cfg.n_queues,
                   dynamic_dma_scratch_size=cfg.dma_scratch)
    f16, f32 = dt.float16, dt.float32

    XT = nc.dram_tensor("xt", [IN_DIM, cfg.n_nodes_pad], f16, kind="ExternalInput")
    WT = nc.dram_tensor("wt", [IN_DIM, OUT_DIM], f16, kind="ExternalInput")
    BR = nc.dram_tensor("brow", [1, OUT_DIM], f16, kind="ExternalInput")
    IO = nc.dram_tensor("iota", [P, P], f16, kind="ExternalInput")
    ID = nc.dram_tensor("ident", [P, P], f16, kind="ExternalInput")
    CO = nc.dram_tensor("cols", [P, sched["tot8"]], dt.int16, kind="ExternalInput")
    L8 = nc.dram_tensor("l8", [P, sched["tot8"]], f16, kind="ExternalInput")
    V8 = nc.dram_tensor("v8", [P, sched["tot8"]], f16, kind="ExternalInput")
    DG = nc.dram_tensor("deg", [1, cfg.rows_pad], f16, kind="ExternalInput")
    OUT = nc.dram_tensor("out", [cfg.rows_pad, OUT_DIM], f16, kind="ExternalOutput")

    T = sched["T"]
    blocks = sched["blocks"]
    sec_meta = sched["sec_meta"]
    nsl_max = sched["nsl_max"]
    nblk = len(blocks)
    t_c = cfg.t_c
    SLAB = cfg.slab
    QUAD = 4

    nslab = t_c // SLAB

    with tile.TileContext(nc) as tc:
        with (
            tc.tile_pool(name="dram", bufs=1, space="DRAM") as dpool,
            tc.tile_pool(name="const", bufs=1) as cpool,
            tc.tile_pool(name="acc", bufs=1) as accp,
            tc.tile_pool(name="xt", bufs=2) as xtp,
            tc.tile_pool(name="hpsum", bufs=2, space="PSUM") as hpp,
            tc.tile_pool(name="hstg", bufs=2) as hsp,
            tc.tile_pool(name="cob", bufs=nblk + 2) as cop,
            tc.tile_pool(name="lvb", bufs=8) as lvp,
            tc.tile_pool(name="degb", bufs=4) as degp,
            tc.tile_pool(name="st", bufs=cfg.st_bufs) as stp,
            tc.tile_pool(name="mt", bufs=cfg.mt_bufs) as mtp,
            tc.tile_pool(name="spsum", bufs=4, space="PSUM") as spp,
        ):
            H = [dpool.tile([P, t_c, OUT_DIM], f16, tag=f"h{c}", name=f"hbuf{c}")
                 for c in range(NCH)]
            H_flat = [h[:].rearrange("p t f -> (p t) f") for h in H]

            wt0 = cpool.tile([P, OUT_DIM], f16)
            wt1 = cpool.tile([P, OUT_DIM], f16)
            nc.sync.dma_start(out=wt0[:], in_=WT[0:P, :])
            nc.sync.dma_start(out=wt1[:], in_=WT[P:2 * P, :])
            brow_t = cpool.tile([1, OUT_DIM], f16)
            nc.sync.dma_start(out=brow_t[:], in_=BR[:, :])
            iota_t = cpool.tile([P, P], f16)
            nc.sync.dma_start(out=iota_t[:], in_=IO[:, :])
            ident_t = cpool.tile([P, P], f16)
            nc.sync.dma_start(out=ident_t[:], in_=ID[:, :])

            # per-block fp16 accumulators, alive for the whole phase-2 span
            acc = {}
            for bi, blk in enumerate(blocks):
                acc[bi] = accp.tile([P, len(blk), OUT_DIM], f16, tag=f"acc{bi}",
                                    name=f"acc{bi}")

            state = dict(gq=0)
            side = {}

            def emit_sidebands(cc):
                """Sideband loads for round cc (gpsimd queue; must be emitted
                after round cc-1's gathers to keep the gpsimd stream
                acyclic under pool-rotation waits)."""
                for bi in range(nblk):
                    sm = sec_meta[cc * nblk + bi]
                    nsl = sm["nsl"]
                    co_t = cop.tile([P, nsl_max * 8], dt.int16, tag="co")
                    l8_t = lvp.tile([P, nsl_max * 8], f16, tag="l8")
                    v8_t = lvp.tile([P, nsl_max * 8], f16, tag="v8")
                    s0 = sm["slot0"]
                    nc.gpsimd.dma_start(out=co_t[:, :nsl * 8],
                                        in_=CO[:, s0 * 8:(s0 + nsl) * 8])
                    nc.gpsimd.dma_start(out=l8_t[:, :nsl * 8],
                                        in_=L8[:, s0 * 8:(s0 + nsl) * 8])
                    nc.gpsimd.dma_start(out=v8_t[:, :nsl * 8],
                                        in_=V8[:, s0 * 8:(s0 + nsl) * 8])
                    side[(cc, bi)] = (co_t, l8_t, v8_t, sm)

            def emit_slab(cc, s0):
                """One projection slab of chunk cc."""
                base = cc * t_c + s0
                xt0 = xtp.tile([P, SLAB * P], f16, tag="x0")
                xt1 = xtp.tile([P, SLAB * P], f16, tag="x1")
                nc.sync.dma_start(out=xt0[:],
                                  in_=XT[0:P, base * P:(base + SLAB) * P])
                nc.sync.dma_start(out=xt1[:],
                                  in_=XT[P:2 * P, base * P:(base + SLAB) * P])
                hst = hsp.tile([P, SLAB, OUT_DIM], f16)
                for q0 in range(0, SLAB, QUAD):
                    nq = min(QUAD, SLAB - q0)
                    ps = hpp.tile([P, QUAD * OUT_DIM], f32)
                    for qi in range(nq):
                        o = (q0 + qi) * P
                        seg = ps[:, qi * OUT_DIM:(qi + 1) * OUT_DIM]
                        nc.tensor.matmul(seg, lhsT=xt0[:, o:o + P], rhs=wt0[:],
                                         start=True, stop=False)
                        nc.tensor.matmul(seg, lhsT=xt1[:, o:o + P], rhs=wt1[:],
                                         start=False, stop=True)
                    nc.scalar.copy(
                        hst[:, q0:q0 + nq, :],
                        ps[:, :nq * OUT_DIM].rearrange("p (q f) -> p q f",
                                                       f=OUT_DIM))
                nc.scalar.dma_start(out=H[cc][:, s0:s0 + SLAB, :], in_=hst[:])

            def emit_block(cc, bi):
                """Phase-2 work of round cc, block bi: selector build,
                gathers, matmul chains, accumulator evict."""
                blk = blocks[bi]
                nbg = len(blk)
                co_t, l8_t, v8_t, sm = side[(cc, bi)]
                nsl = sm["nsl"]
                o_g = sm["o"]

                # selector build with val folded in (DVE 2x packed mode)
                st = stp.tile([P, nsl_max * P], f16, tag="st")
                st4 = st[:, :nsl * P].rearrange("p (s j i) -> p s j i",
                                                j=16, i=8)
                l84 = l8_t[:, :nsl * 8].rearrange("p (s i) -> p s i", i=8)[
                    :, :, None, :].to_broadcast([P, nsl, 16, 8])
                v84 = v8_t[:, :nsl * 8].rearrange("p (s i) -> p s i", i=8)[
                    :, :, None, :].to_broadcast([P, nsl, 16, 8])
                io4 = iota_t[:].rearrange("p (j i) -> p j i", i=8)[
                    :, None, :, :].to_broadcast([P, nsl, 16, 8])
                nc.vector.tensor_tensor(out=st4, in0=l84, in1=io4,
                                        op=mybir.AluOpType.is_equal)
                nc.vector.tensor_tensor(out=st4, in0=st4, in1=v84,
                                        op=mybir.AluOpType.mult)

                # gathers, one call per (group, chunk) bucket
                trim = cc * nblk + bi >= sched["fresh_skip"]
                mt = mtp.tile([P, nsl_max, OUT_DIM], f16, tag="m")
                for gi in blk:
                    tg = int(T[gi, cc])
                    a = o_g[gi]
                    nvalid = int(sched["M"][gi, cc]) if trim else tg * P
                    nc.gpsimd.dma_gather(
                        out_ap=mt[:, a:a + tg, :],
                        in_ap=H_flat[cc],
                        idxs_ap=co_t[:, a * 8:(a + tg) * 8],
                        num_idxs=tg * P,
                        num_idxs_reg=nvalid,
                        elem_size=OUT_DIM,
                        queue_num=state["gq"] % cfg.n_queues,
                    )
                    state["gq"] += 1

                if cc == 0:
                    deg_t = degp.tile([1, cfg.block_groups * P], f16, tag="deg")
                    g0 = blk[0]
                    nc.gpsimd.dma_start(out=deg_t[:, :nbg * P],
                                        in_=DG[:, g0 * P:(g0 + nbg) * P])

                # matmuls: per group, bias/acc re-inject + selector chain
                for pi in range(0, nbg, 4):
                    npg = min(4, nbg - pi)
                    ps = spp.tile([P, 4 * OUT_DIM], f32)
                    for j in range(npg):
                        gi = blk[pi + j]
                        seg = ps[:, j * OUT_DIM:(j + 1) * OUT_DIM]
                        if cc == 0:
                            jj = pi + j
                            nc.tensor.matmul(
                                seg,
                                lhsT=deg_t[0:1, jj * P:(jj + 1) * P],
                                rhs=brow_t[0:1, :],
                                start=True, stop=False)
                        else:
                            nc.tensor.matmul(
                                seg, lhsT=ident_t[:],
                                rhs=acc[bi][:, pi + j, :],
                                start=True, stop=False)
                        tg = int(T[gi, cc])
                        a = o_g[gi]
                        for k in range(tg):
                            nc.tensor.matmul(
                                seg,
                                lhsT=st[:, (a + k) * P:(a + k + 1) * P],
                                rhs=mt[:, a + k, :],
                                start=False, stop=k == tg - 1)
                    # evict back to fp16 accumulator
                    nc.scalar.copy(
                        acc[bi][:, pi:pi + npg, :],
                        ps[:, :npg * OUT_DIM].rearrange(
                            "p (q f) -> p q f", f=OUT_DIM))

            # ---------------- emission: round cc's phase-2 interleaves with
            # the projection of chunk cc+1 so the tensor engine never idles.
            emit_sidebands(0)
            for s in range(nslab):
                emit_slab(0, s * SLAB)
            for cc in range(NCH):
                last = cc == NCH - 1
                for i in range(max(nblk, 0 if last else nslab)):
                    if not last and i < nslab:
                        emit_slab(cc + 1, i * SLAB)
                    if i < nblk:
                        emit_block(cc, i)
                if not last:
                    emit_sidebands(cc + 1)

            # ---------------- final output writes
            for bi, blk in enumerate(blocks):
                nbg = len(blk)
                g0 = blk[0]
                nc.sync.dma_start(
                    out=OUT[g0 * P:(g0 + nbg) * P, :]
                    .rearrange("(q p) f -> p q f", p=P),
                    in_=acc[bi][:, :nbg, :])

    nc.finalize()
    return nc


# ---------------------------------------------------------------- driver
_CACHE = {}


def _patch_walrus_ldw_opt():
    """Enable walrus LDWEIGHTS double-buffering (off by default)."""
    import concourse.bass_utils as _bu
    if getattr(_bu, "_ldw_patched", False):
        return
    orig = _bu.run_command

    def run_command2(argv, **kw):
        argv = ["--enable-ldw-opt=true" if a == "--enable-ldw-opt=false" else a
                for a in argv]
        return orig(argv, **kw)

    _bu.run_command = run_command2
    _bu._ldw_patched = True


def _ensure_ntff_hook():
    """Provide antenv.axon_hooks + the ctypes NTFF profile hook when the
    agent image lacks them (needed only for trace=True)."""
    import sys
    import types
    import contextlib
    import ctypes
    try:
        from antenv.axon_hooks import get_axon_ntff_profile_hook  # noqa: F401
        return
    except ImportError:
        pass
    import antenv
    mod = types.ModuleType("antenv.axon_hooks")
    mod._hook = None

    def set_axon_ntff_profile_hook(h):
        mod._hook = h

    def get_axon_ntff_profile_hook():
        return mod._hook

    mod.set_axon_ntff_profile_hook = set_axon_ntff_profile_hook
    mod.get_axon_ntff_profile_hook = get_axon_ntff_profile_hook
    sys.modules["antenv.axon_hooks"] = mod
    antenv.axon_hooks = mod

    so_path = "/opt/axon/libaxon_pjrt.so"
    try:
        lib = ctypes.CDLL(so_path)
    except OSError:
        return
    if not hasattr(lib, "axon_start_nrt_profile"):
        return
    lib.axon_start_nrt_profile.argtypes = [ctypes.POINTER(ctypes.c_int64),
                                           ctypes.c_size_t]
    lib.axon_start_nrt_profile.restype = ctypes.c_int64
    lib.axon_stop_nrt_profile.argtypes = [ctypes.c_char_p]
    lib.axon_stop_nrt_profile.restype = ctypes.c_int64

    @contextlib.contextmanager
    def _hook(output_dir, device_ids):
        import jax
        jax.devices()
        if device_ids:
            ids = (ctypes.c_int64 * len(device_ids))(*device_ids)
            rc = lib.axon_start_nrt_profile(ids, len(device_ids))
        else:
            rc = lib.axon_start_nrt_profile(None, 0)
        if rc != 0:
            raise RuntimeError(f"axon_start_nrt_profile rc={rc}")
        try:
            yield
        finally:
            n = lib.axon_stop_nrt_profile(str(output_dir).encode())
            print(f"ntff profile: {n} file(s) written to {output_dir}", flush=True)

    set_axon_ntff_profile_hook(_hook)


def _prep_inputs(cfg, X, W, b, sched, data):
    Xf = np.asarray(X, np.float32).astype(np.float16)
    n = Xf.shape[0]
    xt = np.zeros((IN_DIM, cfg.n_nodes_pad), dtype=np.float16)
    narr = np.arange(n)
    cch = narr % NCH
    qq = narr // NCH
    col = (cch * cfg.t_c + qq // P) * P + (qq % P)
    xt[:, col] = Xf.T
    wt = np.asarray(W, np.float32).T.astype(np.float16)
    brow = np.asarray(b, np.float32).astype(np.float16)[None, :]
    iota = np.tile(np.arange(P, dtype=np.float16)[None, :], (P, 1))
    ident = np.eye(P, dtype=np.float16)
    in_maps = []
    for ci in range(N_CORES):
        in_maps.append({
            "xt": xt, "wt": wt, "brow": brow, "iota": iota, "ident": ident,
            "cols": data["cols"][ci], "l8": data["l8"][ci],
            "v8": data["v8"][ci], "deg": data["deg"][ci],
        })
    return in_maps


def run(X, edge_row, edge_col, edge_val, W, b, cfg=DEFAULT_CFG, trace=False):
    if trace:
        _ensure_ntff_hook()
    sched, data = build_schedule(cfg, edge_row, edge_col, edge_val)
    key = ("prog", cfg.n_nodes, cfg.rows_per_core, sched["tot_slots"],
           tuple(sched["T"].reshape(-1)), tuple(sched["M"].reshape(-1)))
    if key not in _CACHE:
        _CACHE.clear()
        _CACHE[key] = build_program(cfg, sched)
    nc = _CACHE[key]
    in_maps = _prep_inputs(cfg, X, W, b, sched, data)
    res = run_bass_kernel_spmd(nc, in_maps, core_ids=list(range(N_CORES)),
                               trace=trace)
    outs = [res.results[ci]["out"][:cfg.rows_per_core] for ci in range(N_CORES)]
    full = np.concatenate(outs, axis=0).astype(np.float32)
    return full, res


def kernel(X, edge_row, edge_col, edge_val, W, b):
    out, _ = run(X, edge_row, edge_col, edge_val, W, b)
    return out
